# revision 53
# baseline (speedup 1.0000x reference)
"""AttentionGNN (3-layer TransformerConv) Trainium2 kernel.

  - Nodes partitioned across 8 cores by dst range (12500 each); edges routed to
    the core owning their destination.
  - Math restructure: scores = <q~[dst], x[src]> with q~ = (x Wq + bq) Wk^T/sqrt(C)
    (the bk term is a per-dst softmax constant -> cancels; segment-max dropped --
    scores are in [-2, 2.1]).  out = (sum w x[src]) / (sum w) @ Wv + (bv+bs) + x Ws.
  - Edge phase: slots sorted by (src quarter, dst) and PAIRED by dst: a pair of
    adjacent slot-columns shares one dst, so the q~ gather and the scatter-add
    run at pair granularity (half the DMA indices).  Pad slots get a -30 score
    bias (exp -> ~0) so their payload and denominator contributions vanish.
  - Aggregation accumulates in SBUF via parity-split dma_scatter_add
    (tokens_per_rank=128: dst d lands on partition d&127, col (d>>7)>>1, buffer
    (d>>7)&1).  Phase C reads agg tiles straight out of SBUF.
  - Dense phases on PE with DMAs batched 4 tiles at a time, stores on the ACT
    HWDGE queue to unclog the SP queue.
  - Host mediates inter-layer exchange (3 SPMD launches).
"""

import math
import os

import numpy as np

N_NODES = 100000
N_EDGES = 1600000
NCORES = 8
NL = N_NODES // NCORES          # 12500
P = 128
J = (NL + P - 1) // P           # 98
NJ = P * J                      # 12544
QCH = 4                         # src quarters
QSZ = N_NODES // QCH            # 25000
MC = 64                         # slot columns per chunk
MCP = MC // 2                   # pair columns per chunk
CHUNK_SLOTS = P * MC            # 8192
CHUNK_PAIRS = P * MCP           # 4096
NSUB_X = 8                      # x-gather sub-calls (1024-desc SWDGE carveout)
NSUB_P = 4                      # q-gather / scatter sub-calls (pairs)
CD = 64                         # unified feature width (layer0 zero-padded)
Cp = CD + 1
BIAS = -30.0
NG = 50                         # agg group columns (ceil((J+1)/2))

_PLAN_CACHE = {}


def _wrap_idx(lst, ncols):
    """int16 list -> [128, ncols] wrapped-in-16 + replicated-across-groups."""
    n = len(lst)
    out = np.zeros((P, ncols), np.int16)
    cols = (n + 15) // 16
    pad = np.zeros(cols * 16 - n, np.int16)
    w = np.concatenate([lst.astype(np.int16), pad]).reshape(cols, 16).T
    for g in range(8):
        out[g * 16:(g + 1) * 16, :cols] = w
    return out


def _build_plan(edge_index):
    key = hash(edge_index.tobytes())
    if key in _PLAN_CACHE:
        return _PLAN_CACHE[key]

    src = np.ascontiguousarray(edge_index[0]).astype(np.int64)
    dst = np.ascontiguousarray(edge_index[1]).astype(np.int64)

    cores = []
    for c in range(NCORES):
        lo, hi = c * NL, (c + 1) * NL
        esel = np.where((dst >= lo) & (dst < hi))[0]
        csrc = src[esel]
        cdst = (dst[esel] - lo).astype(np.int64)
        q = csrc // QSZ
        order = np.lexsort((cdst, q))
        csrc, cdst, q = csrc[order], cdst[order], q[order]

        regions = []   # per quarter: (nchunk_r, xi_slots, pair_dst, bias_slots)
        for r in range(QCH):
            sel = q == r
            ls = (csrc[sel] - r * QSZ).astype(np.int64)
            ld = cdst[sel]
            n = len(ls)
            # pair up edges within each dst run (ld sorted)
            starts = np.searchsorted(ld, ld)          # first occurrence index
            within = np.arange(n) - starts            # rank within the run
            is_first = (within % 2) == 0
            pdst = ld[is_first]                       # dst per pair
            pocc = (within // 2)[is_first]            # pair rank within dst run
            npair = len(pdst)
            nchunk_r = max(1, -(-npair // CHUNK_PAIRS))
            NPr = nchunk_r * CHUNK_PAIRS
            NBr = NPr // 1024                         # scatter sub-call buckets
            assert pocc.max(initial=0) < NBr, (pocc.max(), NBr)
            # bucket assignment: pairs of one dst go to distinct buckets
            # (CCE dedup within a scatter call); keep loads balanced by
            # giving each dst its least-loaded buckets.
            udst, ustart, ucnt = np.unique(pdst, return_index=True,
                                           return_counts=True)
            loads = np.zeros(NBr, np.int64)
            bucket = np.empty(npair, np.int64)
            for ui in range(len(udst)):
                kd = int(ucnt[ui])
                bsel = np.argpartition(loads, kd - 1)[:kd]
                bucket[ustart[ui]:ustart[ui] + kd] = bsel
                loads[bsel] += 1
            assert loads.max(initial=0) <= 1024, (loads.max(), NBr)
            border = np.lexsort((pdst, bucket))
            bcnt = np.bincount(bucket, minlength=NBr)
            bstart = np.zeros(NBr + 1, np.int64)
            np.cumsum(bcnt, out=bstart[1:])
            pos = np.empty(npair, np.int64)           # pair -> linear pair slot
            pos[border] = bucket[border] * 1024 + (
                np.arange(npair) - bstart[bucket[border]])
            # per-pair tables
            p_dst = np.full(NPr, NJ, np.int64)        # dump for pad pairs
            p_e = np.full((NPr, 2), -1, np.int64)     # edge ids per pair half
            p_dst[pos] = pdst
            pair_id_dense = np.cumsum(is_first) - 1   # pair ordinal per edge
            e_pair_global = pos[pair_id_dense]
            p_e[e_pair_global, within % 2] = np.arange(n)
            # pair linear slot -> the two slot indices it owns
            jj = np.arange(NPr)
            ch = jj // CHUNK_PAIRS
            jploc = jj % CHUNK_PAIRS
            col = jploc // P
            pp = jploc % P
            i0 = ch * CHUNK_SLOTS + 2 * col * P + pp
            i1 = i0 + P
            nslots = NPr * 2
            xi_sl = np.zeros(nslots, np.int64)
            bias_sl = np.full(nslots, BIAS, np.float32)
            for hf, ii in ((0, i0), (1, i1)):
                e = p_e[:, hf]
                has = e >= 0
                xi_sl[ii[has]] = ls[e[has]]
                bias_sl[ii[has]] = 0.0
            regions.append((nchunk_r, xi_sl, p_dst, bias_sl))

        nchunk = sum(rg[0] for rg in regions)
        M = nchunk * MC
        xi = np.zeros((P, 8 * M), np.int16)
        qi = np.zeros((P, 4 * M), np.int16)
        si = np.zeros((P, 4 * M), np.int16)
        mb = np.zeros((P, M), np.float32)
        rtag = []
        k0 = 0
        for r, (nchunk_r, xi_sl, p_dst, bias_sl) in enumerate(regions):
            rtag += [r] * nchunk_r
            sl = slice(k0 * 8 * MC, (k0 + nchunk_r) * 8 * MC)
            xi[:, sl] = _wrap_idx(xi_sl, 8 * MC * nchunk_r)
            slp = slice(k0 * 4 * MC, (k0 + nchunk_r) * 4 * MC)
            qi[:, slp] = _wrap_idx(p_dst, 4 * MC * nchunk_r)
            si[:, slp] = _wrap_idx(p_dst, 4 * MC * nchunk_r)
            # bias: slot i -> partition i%128, col i//128
            nslots = nchunk_r * CHUNK_SLOTS
            mb[:, k0 * MC:(k0 + nchunk_r) * MC] = (
                bias_sl.reshape(nslots // P, P).T)
            k0 += nchunk_r
        cores.append(dict(xi=xi, qi=qi, si=si, mb=mb, nchunk=nchunk,
                          rtag=rtag))

    # all cores share one program: pad each quarter's chunk count to the
    # per-quarter max across cores so region tags align.
    nck = np.zeros((NCORES, QCH), np.int64)
    for ci, cc in enumerate(cores):
        for r in cc["rtag"]:
            nck[ci, r] += 1
    ncq = nck.max(axis=0)           # unified chunks per quarter
    nchunk_u = int(ncq.sum())
    rtag_u = sum(([r] * int(ncq[r]) for r in range(QCH)), [])
    M_u = nchunk_u * MC
    for cc in cores:
        xi2 = np.zeros((P, 8 * M_u), np.int16)
        qi2 = np.full((P, 4 * M_u), NJ, np.int16)
        si2 = np.full((P, 4 * M_u), NJ, np.int16)
        mb2 = np.full((P, M_u), BIAS, np.float32)
        src_k = 0
        dst_k = 0
        for r in range(QCH):
            n_r = int(np.sum(np.array(cc["rtag"]) == r))
            xi2[:, dst_k * 8 * MC:(dst_k + n_r) * 8 * MC] = \
                cc["xi"][:, src_k * 8 * MC:(src_k + n_r) * 8 * MC]
            qi2[:, dst_k * 4 * MC:(dst_k + n_r) * 4 * MC] = \
                cc["qi"][:, src_k * 4 * MC:(src_k + n_r) * 4 * MC]
            si2[:, dst_k * 4 * MC:(dst_k + n_r) * 4 * MC] = \
                cc["si"][:, src_k * 4 * MC:(src_k + n_r) * 4 * MC]
            mb2[:, dst_k * MC:(dst_k + n_r) * MC] = \
                cc["mb"][:, src_k * MC:(src_k + n_r) * MC]
            # pad chunks for this quarter keep dump q/s idx (NJ) and BIAS;
            # x idx 0 is fine (bias kills it).
            src_k += n_r
            dst_k += int(ncq[r])
        # merge per-chunk idx streams into one tensor: [xi | qi | si]
        xqs = np.empty((P, 16 * M_u), np.int16)
        for k in range(nchunk_u):
            b = k * 16 * MC
            xqs[:, b:b + 8 * MC] = xi2[:, k * 8 * MC:(k + 1) * 8 * MC]
            xqs[:, b + 8 * MC:b + 12 * MC] = qi2[:, k * 4 * MC:(k + 1) * 4 * MC]
            xqs[:, b + 12 * MC:b + 16 * MC] = si2[:, k * 4 * MC:(k + 1) * 4 * MC]
        cc["xqs"], cc["mb"] = xqs, mb2

    plan = dict(M=M_u, nchunk=nchunk_u, rtag=rtag_u,
                cores=[dict(xqs=cc["xqs"], mb=cc["mb"]) for cc in cores])
    _PLAN_CACHE[key] = plan
    return plan


def _fold_weights(inp, li):
    Wq, bq = np.float64(inp[f"Wq{li}"]), np.float64(inp[f"bq{li}"])
    Wk = np.float64(inp[f"Wk{li}"])
    Wv, bv = np.float64(inp[f"Wv{li}"]), np.float64(inp[f"bv{li}"])
    Ws, bs = np.float64(inp[f"Ws{li}"]), np.float64(inp[f"bs{li}"])
    C = Wq.shape[1]
    Cin = Wq.shape[0]
    A = Wq @ Wk.T / math.sqrt(C)
    a0 = bq @ Wk.T / math.sqrt(C)
    A_aug = np.zeros((CD + 1, CD), np.float32)
    A_aug[:Cin, :Cin] = A
    A_aug[CD, :Cin] = a0
    Cout = Wv.shape[1]
    Wvp = np.zeros((CD, Cout), np.float32)
    Wvp[:Cin] = Wv
    Ws_aug = np.zeros((CD + 1, Cout), np.float32)
    Ws_aug[:Cin] = Ws
    Ws_aug[CD] = bv + bs
    return A_aug, Wvp, Ws_aug


def _build_layer_program(Cout, M, rtag, relu, Cs=CD):
    from contextlib import ExitStack

    import concourse.tile as tile
    from concourse import bacc, mybir
    from concourse.masks import make_identity

    f32 = mybir.dt.float32
    i16 = mybir.dt.int16
    nchunk = M // MC

    nc = bacc.Bacc("TRN2", target_bir_lowering=False, debug=False,
                   num_devices=NCORES)

    xtab = nc.dram_tensor("xtab", [N_NODES, CD], f32, kind="ExternalInput").ap()
    xpt = nc.dram_tensor("xpt", [CD + 1, NJ], f32, kind="ExternalInput").ap()
    xqsd = nc.dram_tensor("xqs", [P, 16 * M], i16, kind="ExternalInput").ap()
    mbd = nc.dram_tensor("mb", [P, M], f32, kind="ExternalInput").ap()
    Aaug = nc.dram_tensor("Aaug", [CD + 1, CD], f32, kind="ExternalInput").ap()
    Wv = nc.dram_tensor("Wv", [CD, Cout], f32, kind="ExternalInput").ap()
    WsA = nc.dram_tensor("WsA", [CD + 1, Cout], f32, kind="ExternalInput").ap()

    qtab = nc.dram_tensor("qtab", [NJ + 1, CD], f32).ap()
    aggd = nc.dram_tensor("aggd", [NJ + P, 2 * CD], f32).ap()
    out = nc.dram_tensor("out", [NJ, Cout], f32, kind="ExternalOutput").ap()

    with tile.TileContext(nc) as tc, ExitStack() as ctx:
        consts = ctx.enter_context(tc.tile_pool(name="consts", bufs=1))
        Asb = consts.tile([CD + 1, CD], f32)
        nc.sync.dma_start(Asb[:], Aaug[:])
        Wvsb = consts.tile([CD, Cout], f32)
        nc.sync.dma_start(Wvsb[:], Wv[:])
        WsAsb = consts.tile([CD + 1, Cout], f32)
        nc.sync.dma_start(WsAsb[:], WsA[:])
        ident = consts.tile([P, P], f32)
        make_identity(nc, ident[:])

        # zero-init aggd (DRAM accumulator; rows NL..NJ stay denom 0 -> NaN in
        # phase C, confined to out rows >= NL which the host drops) and the
        # q~ pad row.  Memset on ACT, init DMA on the PE queue -- both idle
        # early, so this hides under phase A.
        zp = ExitStack()
        zpool = zp.enter_context(tc.tile_pool(name="zeros", bufs=1))
        NRA = NJ + P
        zcols = NRA * 2 * CD // P          # 12672
        NZ = 12                            # init DMA chunks
        zck = zcols // NZ                  # 1056
        zt = zpool.tile([P, zck], f32)
        nc.vector.memset(zt[:], 0.0)
        aggflat = aggd.rearrange("(p r) c -> p (r c)", p=P)
        for z in range(NZ):
            eng = nc.scalar if z % 2 == 0 else nc.sync
            eng.dma_start(aggflat[:, z * zck:(z + 1) * zck], zt[:])
        zq = zpool.tile([1, CD], f32)
        nc.vector.memset(zq[:], 0.0)
        nc.scalar.dma_start(qtab[NJ:NJ + 1, :], zq[:])
        zp.close()

        # ---- phase A: q~ table (batched: 4 tiles per DMA) ----
        pa = ExitStack()
        pa_x = pa.enter_context(tc.tile_pool(name="pa_x", bufs=3))
        pa_ps = pa.enter_context(tc.tile_pool(name="pa_ps", bufs=4, space="PSUM"))
        pa_q = pa.enter_context(tc.tile_pool(name="pa_q", bufs=3))
        t0 = 0
        gi = 0
        while t0 < J:
            G = min(8, J - t0)
            ld_eng, st_eng = ((nc.sync, nc.scalar) if gi % 2 == 0
                              else (nc.scalar, nc.sync))
            xt = pa_x.tile([CD + 1, G * P], f32, tag=f"xt{G}")
            ld_eng.dma_start(xt[:], xpt[:, t0 * P:(t0 + G) * P])
            qsb = pa_q.tile([P, G * CD], f32, tag=f"qsb{G}")
            for g in range(G):
                psq = pa_ps.tile([P, CD], f32)
                nc.tensor.matmul(psq[:], lhsT=xt[:, g * P:(g + 1) * P],
                                 rhs=Asb[:], start=True, stop=True)
                # PSUM reads are DVE/ACT-only (GPSIMD cannot access PSUM)
                nc.vector.tensor_copy(qsb[:, g * CD:(g + 1) * CD], psq[:])
            st_eng.dma_start(
                qtab[t0 * P:(t0 + G) * P, :].rearrange("(g p) c -> p g c", p=P),
                qsb[:].rearrange("p (g c) -> p g c", g=G))
            t0 += G
            gi += 1

        # ---- phase B: edge streaming ----
        pb = ExitStack()
        pXg = pb.enter_context(tc.tile_pool(name="pXg", bufs=3))
        pQg = pb.enter_context(tc.tile_pool(name="pQg", bufs=3))
        pPr = pb.enter_context(tc.tile_pool(name="pPr", bufs=2))
        pAccP = pb.enter_context(tc.tile_pool(name="pAccP", bufs=2))
        psm = pb.enter_context(tc.tile_pool(name="psm", bufs=3))

        for k in range(nchunk):
            r = rtag[k]
            xqst = psm.tile([P, 16 * MC], i16, tag="xqst")
            nc.sync.dma_start(xqst[:], xqsd[:, k * 16 * MC:(k + 1) * 16 * MC])
            xit = xqst[:, :8 * MC]
            qit = xqst[:, 8 * MC:12 * MC]
            sit = xqst[:, 12 * MC:16 * MC]

            Xg = pXg.tile([P, MC, CD], f32, tag="Xg")
            Qg = pQg.tile([P, MCP, CD], f32, tag="Qg")
            cwx = MC // NSUB_X
            iwx = 8 * MC // NSUB_X
            for s in range(NSUB_X):
                nc.gpsimd.dma_gather(
                    out_ap=Xg[:, s * cwx:(s + 1) * cwx, :],
                    in_ap=xtab[r * QSZ:(r + 1) * QSZ, :],
                    idxs_ap=xit[:, s * iwx:(s + 1) * iwx],
                    num_idxs=CHUNK_SLOTS // NSUB_X,
                    num_idxs_reg=CHUNK_SLOTS // NSUB_X,
                    elem_size=CD)
            cwp = MCP // NSUB_P
            iwp = 4 * MC // NSUB_P
            for s in range(NSUB_P):
                nc.gpsimd.dma_gather(
                    out_ap=Qg[:, s * cwp:(s + 1) * cwp, :], in_ap=qtab[:],
                    idxs_ap=qit[:, s * iwp:(s + 1) * iwp],
                    num_idxs=CHUNK_PAIRS // NSUB_P,
                    num_idxs_reg=CHUNK_PAIRS // NSUB_P,
                    elem_size=CD)

            # Pr[:, :, :Cs] = Xg * Qg(pair-broadcast); Pr[:, :, Cs] = bias
            Csp = Cs + 1
            Pr = pPr.tile([P, MC, Csp], f32, tag="Pr")
            nc.sync.dma_start(
                Pr[:, :, Cs].unsqueeze(-1),
                mbd[:, k * MC:(k + 1) * MC].unsqueeze(-1))
            PrV = Pr[:].rearrange("p (m t) c -> p m t c", t=2)
            XgV = Xg[:].rearrange("p (m t) c -> p m t c", t=2)
            nc.vector.tensor_tensor(
                out=PrV[:, :, :, :Cs], in0=XgV[:, :, :, :Cs],
                in1=Qg[:, :, :Cs].unsqueeze(2).to_broadcast([P, MCP, 2, Cs]),
                op=mybir.AluOpType.mult)
            S = psm.tile([P, MC], f32, tag="S")
            nc.vector.tensor_reduce(out=S[:], in_=Pr[:],
                                    axis=mybir.AxisListType.X,
                                    op=mybir.AluOpType.add)
            W = psm.tile([P, MC], f32, tag="W")
            nc.scalar.activation(W[:], S[:], mybir.ActivationFunctionType.Exp)

            # payload reuses the Pr tile (products are dead after the reduce)
            Acc = Pr
            pay_eng = nc.gpsimd if Cs == CD else nc.vector
            pay_eng.tensor_tensor(
                out=Acc[:, :, :Cs], in0=Xg[:, :, :Cs],
                in1=W[:].unsqueeze(-1).to_broadcast([P, MC, Cs]),
                op=mybir.AluOpType.mult)
            nc.scalar.activation(Acc[:, :, Cs], W[:],
                                 mybir.ActivationFunctionType.Copy)

            AccP = pAccP.tile([P, MCP, Csp], f32, tag="AccP")
            AccV = Acc[:].rearrange("p (m t) c -> p m t c", t=2)
            nc.vector.tensor_tensor(
                out=AccP[:], in0=AccV[:, :, 0, :], in1=AccV[:, :, 1, :],
                op=mybir.AluOpType.add)

            for s in range(NSUB_P):
                nc.gpsimd.dma_scatter_add(
                    out_ap=aggd[:, :Csp], in_ap=AccP[:, s * cwp:(s + 1) * cwp, :],
                    idxs_ap=sit[:, s * iwp:(s + 1) * iwp],
                    num_idxs=CHUNK_PAIRS // NSUB_P,
                    num_idxs_reg=CHUNK_PAIRS // NSUB_P,
                    elem_size=Csp, elem_step=2 * CD)
        pb.close()
        pa.close()

        # ---- phase C: normalize + output matmuls (batched) ----
        pc_in = ctx.enter_context(tc.tile_pool(name="pc_in", bufs=3))
        pc_ps = ctx.enter_context(tc.tile_pool(name="pc_ps", bufs=2, space="PSUM"))
        pc_ps2 = ctx.enter_context(tc.tile_pool(name="pc_ps2", bufs=2, space="PSUM"))
        pc_o = ctx.enter_context(tc.tile_pool(name="pc_o", bufs=3))
        t0 = 0
        gi = 0
        while t0 < J:
            G = min(8, J - t0)
            st_eng = nc.scalar if gi % 2 == 0 else nc.sync
            xt2 = pc_in.tile([CD + 1, G * P], f32, tag=f"xt2{G}")
            nc.sync.dma_start(xt2[:], xpt[:, t0 * P:(t0 + G) * P])
            ag = pc_in.tile([P, G, Cs + 1], f32, tag=f"ag{G}")
            nc.scalar.dma_start(
                ag[:], aggd[t0 * P:(t0 + G) * P, :Cs + 1].rearrange(
                    "(g p) c -> p g c", p=P))
            ot = pc_o.tile([P, G * Cout], f32, tag=f"ot{G}")
            # tiles processed in pairs: one PE transpose + one PSUM->SBUF
            # copy covers two dst tiles (lhsT base partition must be 0/32/64,
            # so pairing needs Cs in {32, 64})
            PW = 2 if Cs in (32, 64) else 1
            for gp in range(0, G, PW):
                an2 = pc_in.tile([P, PW * Cs], f32, tag="an2")
                for h in range(PW):
                    g = gp + h
                    rc = pc_in.tile([P, 1], f32, tag="rc")
                    nc.vector.reciprocal(rc[:], ag[:, g, Cs:Cs + 1])
                    ts_eng = nc.vector if h == 0 else nc.gpsimd
                    ts_eng.tensor_scalar_mul(an2[:, h * Cs:(h + 1) * Cs],
                                             ag[:, g, :Cs], rc[:])
                pst = pc_ps.tile([PW * Cs, P], f32, tag="pst")
                nc.tensor.transpose(out=pst[:], in_=an2[:], identity=ident[:])
                ant = pc_in.tile([PW * Cs, P], f32, tag="ant")
                nc.vector.tensor_copy(ant[:], pst[:])
                for h in range(PW):
                    g = gp + h
                    pso = pc_ps2.tile([P, Cout], f32, tag="pso")
                    nc.tensor.matmul(pso[:], lhsT=ant[h * Cs:(h + 1) * Cs, :],
                                     rhs=Wvsb[:Cs, :],
                                     start=True, stop=False,
                                     skip_group_check=True)
                    nc.tensor.matmul(pso[:], lhsT=xt2[:, g * P:(g + 1) * P],
                                     rhs=WsAsb[:], start=False, stop=True,
                                     skip_group_check=True)
                    osl = ot[:, g * Cout:(g + 1) * Cout]
                    fn = (mybir.ActivationFunctionType.Relu if relu
                          else mybir.ActivationFunctionType.Copy)
                    if g % 2 == 0:
                        if relu:
                            nc.vector.tensor_relu(osl, pso[:])
                        else:
                            nc.vector.tensor_copy(osl, pso[:])
                    else:
                        nc.scalar.activation(osl, pso[:], fn)
            st_eng.dma_start(
                out[t0 * P:(t0 + G) * P, :].rearrange("(g p) c -> p g c", p=P),
                ot[:].rearrange("p (g c) -> p g c", g=G))
            t0 += G
            gi += 1

    nc.compile()
    return nc


def _layer_launch(nc, plan, xfull, A_aug, Wv, Ws_aug, sim=False):
    Cin = xfull.shape[1]
    xpad = xfull
    if Cin < CD:
        xpad = np.zeros((N_NODES, CD), np.float32)
        xpad[:, :Cin] = xfull
    in_maps = []
    for c in range(NCORES):
        pc = plan["cores"][c]
        lo = c * NL
        xperm = np.zeros((NJ, CD), np.float32)
        xperm[:NL] = xpad[lo:lo + NL]
        xpt = np.concatenate([xperm.T, np.ones((1, NJ), np.float32)], axis=0)
        in_maps.append({
            "xtab": np.ascontiguousarray(xpad, np.float32),
            "xpt": np.ascontiguousarray(xpt),
            "xqs": pc["xqs"], "mb": pc["mb"],
            "Aaug": A_aug, "Wv": Wv, "WsA": Ws_aug,
        })

    if sim:
        from concourse.bass_interp import CoreSim
        results = []
        for c in range(NCORES if sim == "all" else 1):
            s = CoreSim(nc, trace=bool(int(os.environ.get("GNN_SIMTRACE", "0"))),
                        publish_trace=False,
                        require_finite=False, require_nnan=False)
            for k2, v in in_maps[c].items():
                s.tensor(k2)[:] = v
            s.simulate()
            results.append({"out": np.array(s.tensor("out"))})
            if s.perfetto is not None:
                with open(f"/tmp/sim_trace_c{c}.pftrace", "wb") as f:
                    f.write(s.perfetto.take_serialized())
                print(f"sim trace_time: {s.trace_time} ns")
        return results, None

    from concourse import bass_utils
    trace = bool(int(os.environ.get("GNN_TRACE", "0")))
    br = bass_utils.run_bass_kernel_spmd(
        nc, in_maps, core_ids=list(range(NCORES)), trace=trace)
    return br.results, br


def kernel(**inputs):
    x = np.ascontiguousarray(np.asarray(inputs["x"], np.float32))
    edge_index = np.asarray(inputs["edge_index"])
    plan = _build_plan(edge_index)
    M = plan["M"]

    cfgs = [(8, 64, True), (64, 64, True), (64, 112, False)]
    prog_cache = {}
    sim = os.environ.get("GNN_SIM", "")
    total_ns = 0
    have_ns = True
    h = x
    for li, (Cin, Cout, relu) in enumerate(cfgs):
        pk = (Cin, Cout, relu)
        if pk not in prog_cache:
            prog_cache[pk] = _build_layer_program(Cout, M, plan["rtag"], relu,
                                                  Cs=Cin)
        A_aug, Wv, Ws_aug = _fold_weights(inputs, li)
        results, br = _layer_launch(prog_cache[pk], plan, h, A_aug, Wv, Ws_aug,
                                    sim=sim)
        hn = np.zeros((N_NODES, Cout), np.float32)
        for c in range(len(results)):
            hn[c * NL:(c + 1) * NL] = results[c]["out"][:NL]
        h = hn
        if br is not None and br.exec_time_ns:
            total_ns += br.exec_time_ns
        else:
            have_ns = False

    if have_ns and total_ns:
        kernel.last_exec_ns = total_ns
    return h


kernel.last_exec_ns = None
